# revision 1
# baseline (speedup 1.0000x reference)
"""Trainium2 Bass kernel: windowed-LSTM local attention + linear head (LBNER).

Strategy (v3)
-------------
Data-parallel over batch: B=8 sequences -> 8 NeuronCores, one sequence each.
Feature dim on partitions, L=512 on the free dim.

Per window w in (3,5,7):
  P64 = 64*(Wih @ xT) + 64*bias computed once via fp8-e4m3 DoubleRow matmuls
  (Wih pre-scaled x64, x unscaled; both in the DR [K,2,*] interleave) and
  stored fp8 as [128, dc, gate, L] with gate order i,f,o,g so one sigmoid
  ACT covers i,f,o from a 4-bank PSUM quad.  Step t reads P64 shifted by
  (t - w//2) columns.

  Recurrence (t >= 1): per d-chunk, a 4-bank PSUM quad accumulates
    ident(fp8) @ P64_shift  +  sum_j 64*Whh8[j] @ h8[j]   (fp8 DoubleRow)
  with the running hidden state h8 in fp8 [128, 2, L] pairs.  The DR
  matmuls run full-width (h8 is never column-shifted); only the P-fold and
  post-matmul ops are sliced to the step's valid range, so out-of-range
  window slots never touch state.  ACT applies sigmoid/tanh straight from
  PSUM with scale=1/64.  c stays bf16; h is computed bf16 (hn) then copied
  to fp8 on the Pool engine in a separate pass, so every h8 matmul read
  precedes the same step's writes (true Jacobi, matching the reference).
  The final h per column lands in bf16 `loc` (a column finishes at step
  hw+511-l: late steps write one boundary column, the last step a slab).

The three windows are software-pipelined: A (w=7) steps 1..6 occupy rounds
1..6, C (w=3) rounds 1..2, B (w=5) rounds 3..6, so two windows are always
in flight and one window's matmuls overlap the other's ACT/DVE tails.
Weight/state buffers are per-slot (A=0, C/B share 1 with WAR reuse).

Attention over the 3 window outputs (fp32 x, bf16 locals).  The head runs
as two PSUM groups (fp32 for the x residual term, bf16 for the attention
feature term; walrus rejects mixed-dtype accumulation groups), combined via
an ACT copy + DVE add before the bias activation.
"""

import math
import numpy as np
import ml_dtypes

import concourse.bacc as bacc
import concourse.bass as bass
import concourse.tile as tile
from concourse import mybir
from concourse import bass_utils

B, L, D = 8, 512, 768
NL = 9
WINDOWS = (3, 5, 7)
NW = len(WINDOWS)
G4 = 4 * D          # 3072
P = 128
ND = D // P         # 6 d-chunks
NJ = ND // 2        # 3 k-pair chunks for DoubleRow
NM = G4 // P        # 24 gate-chunks
N_CORES = 8
WS = 64.0           # fp8 weight scale

F32 = mybir.dt.float32
BF16 = mybir.dt.bfloat16
FP8 = mybir.dt.float8e4
AF = mybir.ActivationFunctionType
ALU = mybir.AluOpType
DR = mybir.MatmulPerfMode.DoubleRow


def _gate_ms(dc):
    # m-chunk indices for (i, f, o, g) of d-chunk dc; torch gate order in the
    # 3072 rows is i,f,g,o.
    return (dc, 6 + dc, 18 + dc, 12 + dc)


def _emit(tc, io):
    nc = tc.nc
    from contextlib import ExitStack

    with ExitStack() as ctx:
        const = ctx.enter_context(tc.tile_pool(name="const", bufs=1))
        wihp = ctx.enter_context(tc.tile_pool(name="wihp", bufs=1))
        whhp = ctx.enter_context(tc.tile_pool(name="whhp", bufs=1))
        ppool = ctx.enter_context(tc.tile_pool(name="ppool", bufs=2))
        state = ctx.enter_context(tc.tile_pool(name="state", bufs=1))
        gpool = ctx.enter_context(tc.tile_pool(name="gpool", bufs=2))
        tmp = ctx.enter_context(tc.tile_pool(name="tmp", bufs=2))
        atmp = ctx.enter_context(tc.tile_pool(name="atmp", bufs=4))
        attn = ctx.enter_context(tc.tile_pool(name="attn", bufs=7))
        logp = ctx.enter_context(tc.tile_pool(name="logp", bufs=1))

        # ---- constants / inputs resident in SBUF ----
        # proj inputs (x8, bias) are DMA'd first; xf is only needed at the
        # attention stage, so its transfer rides behind the weight DMAs
        x8 = []   # x.T fp8 in DoubleRow pairs, rhs of the input projections
        for j in range(NJ):
            t_b = const.tile([P, 2, L], FP8, tag=f"x8{j}")
            nc.sync.dma_start(t_b, io["x8"].ap()[j])
            x8.append(t_b)

        # combined LSTM bias * 64, laid out [128, NW, NM]
        bias_sb = const.tile([P, NW, NM], F32, tag="bias")
        nc.sync.dma_start(
            bias_sb, io["bias64"].ap().rearrange("k (m p) -> p k m", p=P)
        )

        xf = []   # x.T fp32, for attention dot + residual head matmul
        for dc in range(ND):
            t_f = const.tile([P, L], F32, tag=f"xf{dc}")
            xf.append(t_f)

        lw = []
        lwb = []
        for dc in range(ND):
            t = const.tile([P, NL], F32, tag=f"lw{dc}")
            lw.append(t)
            tb = const.tile([P, NL], BF16, tag=f"lwb{dc}")
            lwb.append(tb)
        lb_sb = const.tile([NL, 1], F32, tag="lb")

        def load_attn_consts():
            # emitted after the prologue weight DMAs so those go first
            for dc in range(ND):
                nc.sync.dma_start(xf[dc], io["xf"].ap()[dc * P:(dc + 1) * P, :])
                nc.sync.dma_start(lw[dc], io["lwt"].ap()[dc * P:(dc + 1) * P, :])
                nc.sync.dma_start(
                    lwb[dc], io["lwtb"].ap()[dc * P:(dc + 1) * P, :]
                )
            nc.sync.dma_start(
                lb_sb, io["lb"].ap().rearrange("(c o) -> c o", o=1)
            )

        ident_sb = const.tile([P, P], FP8, tag="ident")
        nc.sync.dma_start(ident_sb, io["ident"].ap())

        ones_col = const.tile([P, 1], BF16, tag="ones_col")
        nc.vector.memset(ones_col, 1.0)
        ones_row = const.tile([1, P], BF16, tag="ones_row")
        nc.vector.memset(ones_row, 1.0)

        locs = {}   # per window k: 6 bf16 [128, 512] tiles (final h)
        inv_ws = 1.0 / WS

        # Interleaved schedule: window A (w=7) runs steps 1..6 in rounds 1..6,
        # C (w=3) steps 1..2 in rounds 1..2, B (w=5) steps 1..4 in rounds
        # 3..6.  Two windows are active each round, so one window's matmuls
        # overlap the other's ACT/DVE tails.  State/weight buffers are
        # per-slot: A=slot0, C then B share slot1 (WAR deps serialize reuse).
        WIN = {
            "A": dict(k=2, w=7, slot=0),
            "C": dict(k=0, w=3, slot=1),
            "B": dict(k=1, w=5, slot=1),
        }

        with tc.tile_pool(name="gp", bufs=2, space="PSUM") as gp:

            def win_alloc_proj(W):
                k, slot = W["k"], W["slot"]
                hw0 = W["w"] // 2
                wih8 = []
                for j in range(NJ):
                    t = wihp.tile([P, 2, G4], FP8, tag=f"A{j}")
                    nc.sync.dma_start(t, io["wih8"].ap()[k, j])
                    wih8.append(t)
                whh8 = []
                for j in range(NJ):
                    t = whhp.tile([P, 2, G4], FP8, tag=f"B{slot}{j}")
                    nc.sync.dma_start(t, io["whh8"].ap()[k, j])
                    whh8.append(t)
                W["whh8"] = whh8
                p64t = ppool.tile([P, ND, 4, L], FP8, tag=f"P{slot}")
                W["P64"] = p64t
                c = []
                for dc in range(ND):
                    ct = state.tile([P, L], BF16, tag=f"c{slot}{dc}")
                    nc.gpsimd.memset(ct[:, 0:hw0], 0.0)
                    c.append(ct)
                W["c"] = c
                h8 = []
                for j in range(NJ):
                    ht = state.tile([P, 2, L], FP8, tag=f"h8{slot}{j}")
                    nc.gpsimd.memset(ht[:, :, 0:hw0], 0.0)
                    h8.append(ht)
                W["h8"] = h8
                loc = []
                for dc in range(ND):
                    lt = state.tile([P, L], BF16, tag=f"loc{k}_{dc}")
                    loc.append(lt)
                W["loc"] = loc
                locs[k] = loc

                # input projection
                P64 = W["P64"]
                for dc in range(ND):
                    gm = _gate_ms(dc)
                    q = gp.tile([P, 4, L], F32, tag="q")
                    for gi in range(4):
                        m = gm[gi]
                        for j in range(NJ):
                            nc.tensor.matmul(
                                q[:, gi, :],
                                lhsT=wih8[j][:, :, m * P:(m + 1) * P],
                                rhs=x8[j][:, :, :],
                                start=(j == 0),
                                stop=(j == NJ - 1),
                                perf_mode=DR,
                                skip_group_check=True,
                            )
                    for gi in range(4):
                        m = gm[gi]
                        nc.vector.tensor_scalar_add(
                            P64[:, dc, gi, :], q[:, gi, :],
                            bias_sb[:, k, m:m + 1],
                        )

            def win_step0(W):
                # step 0: h == 0, gates straight from P64 (shifted)
                P64, c, h8 = W["P64"], W["c"], W["h8"]
                hw_ = W["w"] // 2
                s0, e0 = hw_, L
                n0 = e0 - s0
                off0 = -hw_
                hns = []
                for dc in range(ND):
                    # f is unused at step 0 (c starts at 0): two single-gate
                    # sigmoids instead of the i,f,o trio
                    g3 = gpool.tile([P, 3, L], BF16, tag="g3", bufs=4)
                    nc.scalar.activation(
                        g3[:, 0, :n0], P64[:, dc, 0, s0 + off0:e0 + off0],
                        AF.Sigmoid, scale=inv_ws,
                    )
                    nc.scalar.activation(
                        g3[:, 2, :n0], P64[:, dc, 2, s0 + off0:e0 + off0],
                        AF.Sigmoid, scale=inv_ws,
                    )
                    gt = gpool.tile([P, L], BF16, tag="gt", bufs=3)
                    nc.scalar.activation(
                        gt[:, :n0], P64[:, dc, 3, s0 + off0:e0 + off0],
                        AF.Tanh, scale=inv_ws,
                    )
                    nc.vector.tensor_mul(c[dc][:, s0:e0], g3[:, 0, :n0],
                                         gt[:, :n0])
                    tcz = gpool.tile([P, L], BF16, tag="tc", bufs=3)
                    nc.scalar.activation(tcz[:, :n0], c[dc][:, s0:e0], AF.Tanh)
                    hn = tmp.tile([P, L], BF16, tag=f"hn{dc}", bufs=1)
                    nc.vector.tensor_mul(hn[:, :n0], g3[:, 2, :n0], tcz[:, :n0])
                    hns.append(hn)
                for dc in range(ND):
                    nc.gpsimd.tensor_copy(
                        h8[dc // 2][:, dc % 2, s0:e0], hns[dc][:, :n0]
                    )

            def step_ranges(W, t):
                hw_ = W["w"] // 2
                off = t - hw_
                s = max(0, -off)
                e = min(L, L - off)
                return off, s, e, e - s

            def mm_phase(W, t):
                off, s, e, n = step_ranges(W, t)
                P64, whh8, h8 = W["P64"], W["whh8"], W["h8"]
                qs = []
                for dc in range(ND):
                    gm = _gate_ms(dc)
                    q = gp.tile([P, 4, L], F32, tag="q")
                    qs.append(q)
                    for gi in range(4):
                        m = gm[gi]
                        nc.tensor.matmul(
                            q[:, gi, s:e],
                            lhsT=ident_sb[:],
                            rhs=P64[:, dc, gi, s + off:e + off],
                            start=True,
                            stop=False,
                            skip_group_check=True,
                        )
                        for j in range(NJ):
                            nc.tensor.matmul(
                                q[:, gi, :],
                                lhsT=whh8[j][:, :, m * P:(m + 1) * P],
                                rhs=h8[j][:, :, :],
                                start=False,
                                stop=(j == NJ - 1),
                                perf_mode=DR,
                                skip_group_check=True,
                            )
                W["qs"] = qs

            def tail_phase(W, t):
                off, s, e, n = step_ranges(W, t)
                hw_ = W["w"] // 2
                last = (t == W["w"] - 1)
                c, loc, qs = W["c"], W["loc"], W["qs"]
                hns = []
                g3s = {}
                # tanh(c) lags the gate pass by one d-chunk so the ACT queue
                # never head-of-line blocks on the DVE c-update.
                for dc in range(ND + 1):
                    if dc < ND:
                        g3 = gpool.tile([P, 3, L], BF16, tag="g3", bufs=4)
                        nc.scalar.activation(
                            g3[:, :, :n], qs[dc][:, 0:3, s:e], AF.Sigmoid,
                            scale=inv_ws,
                        )
                        g3s[dc] = g3
                        gt = gpool.tile([P, L], BF16, tag="gt", bufs=3)
                        nc.scalar.activation(
                            gt[:, :n], qs[dc][:, 3, s:e], AF.Tanh, scale=inv_ws
                        )
                        t1 = tmp.tile([P, L], BF16, tag="tt", bufs=3)
                        nc.vector.tensor_mul(t1[:, :n], g3[:, 0, :n], gt[:, :n])
                        t2 = tmp.tile([P, L], BF16, tag="tt", bufs=3)
                        nc.vector.tensor_mul(t2[:, :n], g3[:, 1, :n],
                                             c[dc][:, s:e])
                        nc.vector.tensor_add(c[dc][:, s:e], t1[:, :n], t2[:, :n])
                    if dc >= 1:
                        pd = dc - 1
                        g3p = g3s.pop(pd)
                        tcz = gpool.tile([P, L], BF16, tag="tc", bufs=3)
                        nc.scalar.activation(tcz[:, :n], c[pd][:, s:e], AF.Tanh)
                        if last:
                            # final h for all columns still pending
                            nc.vector.tensor_mul(
                                loc[pd][:, s:e], g3p[:, 2, :n], tcz[:, :n]
                            )
                        else:
                            hn = tmp.tile([P, L], BF16, tag=f"hn{pd}", bufs=1)
                            nc.vector.tensor_mul(hn[:, :n], g3p[:, 2, :n],
                                                 tcz[:, :n])
                            hns.append(hn)
                            if t >= hw_:
                                # column e-1 gets its final h at this step
                                nc.vector.tensor_copy(
                                    loc[pd][:, e - 1:e], hn[:, n - 1:n]
                                )
                W["hns"] = hns

            def copy_phase(W, t):
                if t == W["w"] - 1:
                    return
                off, s, e, n = step_ranges(W, t)
                h8, hns = W["h8"], W["hns"]
                for dc in range(ND):
                    nc.gpsimd.tensor_copy(
                        h8[dc // 2][:, dc % 2, s:e], hns[dc][:, :n]
                    )

            A, B, C = WIN["A"], WIN["B"], WIN["C"]
            win_alloc_proj(A)
            win_step0(A)
            win_alloc_proj(C)
            win_step0(C)
            load_attn_consts()
            for r in range(1, 7):
                active = []
                if r <= 6:
                    active.append((A, r))
                if r <= 2:
                    active.append((C, r))
                if r >= 3:
                    active.append((B, r - 2))
                for W, t in active:
                    mm_phase(W, t)
                for W, t in active:
                    tail_phase(W, t)
                for W, t in active:
                    copy_phase(W, t)
                if r == 2:
                    # B's weights/proj/step0 slot in while A round 3 runs
                    win_alloc_proj(B)
                    win_step0(B)

        locs = [locs[0], locs[1], locs[2]]
        # ---- attention over the 3 window outputs ----
        with tc.tile_pool(name="ap2", bufs=1, space="PSUM") as ap2:
            a_sb = []
            for k in range(NW):
                psd = ap2.tile([1, L], F32, tag=f"d{k}")
                for dc in range(ND):
                    td = atmp.tile([P, L], BF16, tag="tf", bufs=2)
                    nc.vector.tensor_mul(td, xf[dc][:], locs[k][dc][:])
                    nc.tensor.matmul(
                        psd,
                        lhsT=ones_col[:],
                        rhs=td[:],
                        start=(dc == 0),
                        stop=(dc == ND - 1),
                    )
                ak = attn.tile([1, L], BF16, tag=f"ak{k}", bufs=1)
                nc.scalar.activation(ak, psd, AF.Copy, scale=1.0 / math.sqrt(D))
                a_sb.append(ak)

            mx1 = attn.tile([1, L], F32, tag="mx", bufs=2)
            nc.vector.tensor_max(mx1, a_sb[0][:], a_sb[1][:])
            mx2 = attn.tile([1, L], F32, tag="mx", bufs=2)
            nc.vector.tensor_max(mx2, mx1[:], a_sb[2][:])
            e_sb = []
            for k in range(NW):
                d_k = attn.tile([1, L], BF16, tag="sm", bufs=5)
                nc.vector.tensor_sub(d_k, a_sb[k][:], mx2[:])
                ek = attn.tile([1, L], BF16, tag="sm", bufs=5)
                nc.scalar.activation(ek, d_k[:], AF.Exp)
                e_sb.append(ek)
            s1 = attn.tile([1, L], F32, tag="mx", bufs=2)
            nc.vector.tensor_add(s1, e_sb[0][:], e_sb[1][:])
            s2 = attn.tile([1, L], F32, tag="mx", bufs=2)
            nc.vector.tensor_add(s2, s1[:], e_sb[2][:])
            r = attn.tile([1, L], F32, tag="mx", bufs=2)
            nc.vector.reciprocal(r, s2[:])

            wb = []   # attention weights broadcast to [128, 512] (PSUM)
            for k in range(NW):
                wk = attn.tile([1, L], BF16, tag="sm", bufs=5)
                nc.vector.tensor_mul(wk, e_sb[k][:], r[:])
                pb = ap2.tile([P, L], F32, tag="bc", bufs=2)
                nc.tensor.matmul(
                    pb, lhsT=ones_row[:], rhs=wk[:], start=True, stop=True,
                )
                wbs = attn.tile([P, L], BF16, tag=f"wbs{k}", bufs=1)
                nc.scalar.activation(wbs, pb, AF.Copy)
                wb.append(wbs)

            # ---- head: logits = lin_w @ (x + sum_k attn_k * locs_k) + b ----
            # two psum groups (walrus rejects mixed-dtype accumulation):
            # fp32 for the x term, bf16 for the attention-feature term
            ps_log = ap2.tile([NL, L], F32, tag="log")
            for dc in range(ND):
                nc.tensor.matmul(
                    ps_log,
                    lhsT=lw[dc][:],
                    rhs=xf[dc][:],
                    start=(dc == 0),
                    stop=(dc == ND - 1),
                )
            ps_log2 = ap2.tile([NL, L], F32, tag="log2")
            for dc in range(ND):
                lf = atmp.tile([P, L], BF16, tag="tg", bufs=4)
                nc.vector.tensor_mul(lf, wb[0][:], locs[0][dc][:])
                t3 = atmp.tile([P, L], BF16, tag="tg", bufs=4)
                nc.vector.tensor_mul(t3, wb[1][:], locs[1][dc][:])
                lf2 = atmp.tile([P, L], BF16, tag="tg", bufs=4)
                nc.vector.tensor_add(lf2, lf[:], t3[:])
                t4 = atmp.tile([P, L], BF16, tag="tg", bufs=4)
                nc.vector.tensor_mul(t4, wb[2][:], locs[2][dc][:])
                lf3 = atmp.tile([P, L], BF16, tag="tg", bufs=4)
                nc.vector.tensor_add(lf3, lf2[:], t4[:])
                nc.tensor.matmul(
                    ps_log2,
                    lhsT=lwb[dc][:],
                    rhs=lf3[:],
                    start=(dc == 0),
                    stop=(dc == ND - 1),
                )
            log2s = logp.tile([NL, L], BF16, tag="log2s")
            nc.scalar.activation(log2s, ps_log2, AF.Copy)
            logsum = logp.tile([NL, L], F32, tag="logsum")
            nc.vector.tensor_add(logsum, ps_log[:], log2s[:])
            logits = logp.tile([NL, L], F32, tag="logits")
            nc.scalar.activation(logits, logsum, AF.Identity, bias=lb_sb[:, 0:1])
            nc.sync.dma_start(io["out"].ap().rearrange("l c -> c l"), logits[:])


_NC_CACHE = {}


def _get_nc():
    if "nc" not in _NC_CACHE:
        nc = bacc.Bacc("TRN2", target_bir_lowering=False, debug=False)
        io = {
            "xf": nc.dram_tensor("xf", [D, L], F32, kind="ExternalInput"),
            "x8": nc.dram_tensor("x8", [NJ, P, 2, L], FP8, kind="ExternalInput"),
            "wih8": nc.dram_tensor("wih8", [NW, NJ, P, 2, G4], FP8, kind="ExternalInput"),
            "whh8": nc.dram_tensor(
                "whh8", [NW, NJ, P, 2, G4], FP8, kind="ExternalInput"
            ),
            "bias64": nc.dram_tensor("bias64", [NW, G4], F32, kind="ExternalInput"),
            "lwt": nc.dram_tensor("lwt", [D, NL], F32, kind="ExternalInput"),
            "lwtb": nc.dram_tensor("lwtb", [D, NL], BF16, kind="ExternalInput"),
            "lb": nc.dram_tensor("lb", [NL], F32, kind="ExternalInput"),
            "ident": nc.dram_tensor("ident", [P, P], FP8, kind="ExternalInput"),
            "out": nc.dram_tensor("out", [L, NL], F32, kind="ExternalOutput"),
        }
        with tile.TileContext(nc) as tc:
            _emit(tc, io)
        nc.compile()
        _NC_CACHE["nc"] = nc
    return _NC_CACHE["nc"]


def _in_maps(sequence_output, W_ih, W_hh, b_ih, b_hh, lin_w, lin_b):
    x = np.asarray(sequence_output, np.float32)
    bf = ml_dtypes.bfloat16
    e4 = ml_dtypes.float8_e4m3
    WihT = np.transpose(np.asarray(W_ih, np.float32), (0, 2, 1))  # [NW, D, G4]
    Wih8 = np.clip(WihT * WS, -240.0, 240.0).astype(e4)
    Wih8 = np.ascontiguousarray(
        Wih8.reshape(NW, NJ, 2, P, G4).transpose(0, 1, 3, 2, 4)
    )
    WhhT = np.transpose(np.asarray(W_hh, np.float32), (0, 2, 1))  # [NW, D, G4]
    Whh8 = np.clip(WhhT * WS, -240.0, 240.0).astype(e4)
    # DoubleRow interleave: [NW, D, G4] -> [NW, NJ, P, 2, G4]
    Whh8 = np.ascontiguousarray(
        Whh8.reshape(NW, NJ, 2, P, G4).transpose(0, 1, 3, 2, 4)
    )
    # proj PSUM is already 64*(Wih@x) (weights pre-scaled), so bias ships x64
    bias64 = WS * (np.asarray(b_ih, np.float32) + np.asarray(b_hh, np.float32))
    lwt = np.ascontiguousarray(np.asarray(lin_w, np.float32).T)
    lb = np.asarray(lin_b, np.float32)
    maps = []
    for b in range(B):
        xT = np.ascontiguousarray(x[b].T)
        x8 = np.clip(xT, -240.0, 240.0).astype(e4)
        x8 = np.ascontiguousarray(
            x8.reshape(NJ, 2, P, L).transpose(0, 2, 1, 3)
        )
        maps.append({
            "xf": xT,
            "x8": x8,
            "wih8": Wih8,
            "whh8": Whh8,
            "bias64": bias64,
            "lwt": lwt,
            "lwtb": lwt.astype(bf),
            "lb": lb,
            "ident": np.eye(P, dtype=np.float32).astype(e4),
        })
    return maps


def kernel(sequence_output, W_ih, W_hh, b_ih, b_hh, lin_w, lin_b):
    nc = _get_nc()
    maps = _in_maps(sequence_output, W_ih, W_hh, b_ih, b_hh, lin_w, lin_b)
    res = bass_utils.run_bass_kernel_spmd(nc, maps, core_ids=list(range(N_CORES)))
    return np.stack([res.results[b]["out"] for b in range(B)], axis=0)


def run_traced(inputs, **kw):
    """For test.py: run with NTFF tracing, returns BassKernelResults."""
    nc = _get_nc()
    maps = _in_maps(**inputs)
    return bass_utils.run_bass_kernel_spmd(
        nc, maps, core_ids=list(range(N_CORES)), trace=True, **kw
    )



# revision 13
# speedup vs baseline: 1.0984x; 1.0984x over previous
"""Trainium2 Bass kernel: windowed-LSTM local attention + linear head (LBNER).

Strategy (v4)
-------------
Data-parallel over batch: B=8 sequences -> 8 NeuronCores, one sequence each.
Feature dim on partitions, L=512 on the free dim.

Per window w in (3,5,7):
  P64 = 64*(Wih @ xT) + 64*bias computed once via fp8-e4m3 DoubleRow matmuls
  (Wih pre-scaled x64, x unscaled; both in the DR [K,2,*] interleave) and
  stored fp8 as [128, dc, gate, L] with gate order i,f,o,g so one sigmoid
  ACT covers i,f,o from a 4-bank PSUM quad.  Step t reads P64 shifted by
  (t - w//2) columns.

  Recurrence (t >= 1): per d-chunk, a 4-bank PSUM quad accumulates
    identDR(fp8) @ P64_pair_shift  +  sum_j 64*Whh8[j] @ h8[j]
  all in fp8 DoubleRow (the P64 re-injection uses gate-pair views of P64
  with component-select DR identities, so it costs 0.5 cyc/row like the
  Whh matmuls).  The DR matmuls run full-width (h8 is never shifted); only
  the P-fold and post-matmul ops are sliced to the step's valid range.
  ACT applies sigmoid/tanh straight from PSUM with scale=1/64.  c stays
  bf16; h8 (fp8) is produced DIRECTLY by a gpsimd tensor_mul
  (o * tanh(c) -> fp8), freeing the DVE and removing one pipeline hop.
  The final h per column lands in bf16 `loc`.

Schedule: A (w=7) steps 1..6 in rounds 1..6, C (w=3) steps 1..2 in rounds
1..2, B (w=5) steps 1..4 in rounds 2..5 (proj B in round 1, step0 B late
round 1), so B finishes before the last round and its attention dots can
overlap round 6.  The x-residual head matmul (lin_w @ x) runs in the
prologue on a borrowed PSUM quad slot; DMAs are ordered so projection A
starts as early as possible.

Attention: dot(x, loc_k) via bf16 DVE muls + ones-vector PE reduction into
one [1, 3, L] PSUM tile; batched softmax; head term as per-window bf16
matmuls Y_k = lin_w @ loc_k combined with broadcast attention weights on
the DVE ([9, L] ops), plus the prologue x-term and bias.
"""

import math
import numpy as np
import ml_dtypes

import concourse.bacc as bacc
import concourse.bass as bass
import concourse.tile as tile
from concourse import mybir
from concourse import bass_utils

B, L, D = 8, 512, 768
NL = 9
WINDOWS = (3, 5, 7)
NW = len(WINDOWS)
G4 = 4 * D          # 3072
P = 128
ND = D // P         # 6 d-chunks
NJ = ND // 2        # 3 k-pair chunks for DoubleRow
NM = G4 // P        # 24 gate-chunks
N_CORES = 8
WS = 64.0           # fp8 weight scale

F32 = mybir.dt.float32
BF16 = mybir.dt.bfloat16
FP8 = mybir.dt.float8e4
AF = mybir.ActivationFunctionType
ALU = mybir.AluOpType
DR = mybir.MatmulPerfMode.DoubleRow


def _gate_ms(dc):
    # m-chunk indices for (i, f, o, g) of d-chunk dc; torch gate order in the
    # 3072 rows is i,f,g,o.
    return (dc, 6 + dc, 18 + dc, 12 + dc)


def _emit(tc, io):
    nc = tc.nc
    from contextlib import ExitStack

    with ExitStack() as ctx:
        const = ctx.enter_context(tc.tile_pool(name="const", bufs=1))
        wihp = ctx.enter_context(tc.tile_pool(name="wihp", bufs=1))
        whhp = ctx.enter_context(tc.tile_pool(name="whhp", bufs=1))
        ppool = ctx.enter_context(tc.tile_pool(name="ppool", bufs=2))
        state = ctx.enter_context(tc.tile_pool(name="state", bufs=1))
        gpool = ctx.enter_context(tc.tile_pool(name="gpool", bufs=2))
        tmp = ctx.enter_context(tc.tile_pool(name="tmp", bufs=2))
        atmp = ctx.enter_context(tc.tile_pool(name="atmp", bufs=4))
        attn = ctx.enter_context(tc.tile_pool(name="attn", bufs=7))
        logp = ctx.enter_context(tc.tile_pool(name="logp", bufs=1))

        # ---- prologue DMAs, ordered for earliest projection start ----
        x8 = []   # x.T fp8 in DoubleRow pairs, rhs of the input projections
        for j in range(NJ):
            t_b = const.tile([P, 2, L], FP8, tag=f"x8{j}")
            nc.sync.dma_start(t_b, io["x8"].ap()[j])
            x8.append(t_b)

        # combined LSTM bias * 64, laid out [128, NW, NM]
        bias_sb = const.tile([P, NW, NM], F32, tag="bias")
        nc.sync.dma_start(
            bias_sb, io["bias64"].ap().rearrange("k (m p) -> p k m", p=P)
        )

        # DR component-select identities: [0]=even (comp0), [1]=odd (comp1)
        ident_sb = []
        for ii in range(2):
            it = const.tile([P, 2, P], FP8, tag=f"identp{ii}")
            nc.sync.dma_start(it, io["identp"].ap()[ii])
            ident_sb.append(it)

        xb = []   # x.T bf16, for attention dots + residual head matmul
        lwb = []
        for dc in range(ND):
            t_b2 = const.tile([P, L], BF16, tag=f"xb{dc}")
            xb.append(t_b2)
            tb = const.tile([P, NL], BF16, tag=f"lwb{dc}")
            lwb.append(tb)
        lb_sb = const.tile([NL, 1], F32, tag="lb")

        ones_col = const.tile([P, 1], BF16, tag="ones_col")
        nc.vector.memset(ones_col, 1.0)
        ones_row = const.tile([1, P], BF16, tag="ones_row")
        nc.vector.memset(ones_row, 1.0)

        locs = {}   # per window k: 6 bf16 [128, 512] tiles (final h)
        inv_ws = 1.0 / WS

        WIN = {
            "A": dict(k=2, w=7, slot=0),
            "C": dict(k=0, w=3, slot=1),
            "B": dict(k=1, w=5, slot=1),
        }

        def win_dma_wih(W):
            wih8 = []
            for j in range(NJ):
                t = wihp.tile([P, 2, G4], FP8, tag=f"A{j}")
                nc.sync.dma_start(t, io["wih8"].ap()[W["k"], j])
                wih8.append(t)
            W["wih8"] = wih8

        def win_dma_whh(W):
            whh8 = []
            for j in range(NJ):
                t = whhp.tile([P, 2, G4], FP8, tag=f"B{W['slot']}{j}")
                nc.sync.dma_start(t, io["whh8"].ap()[W["k"], j])
                whh8.append(t)
            W["whh8"] = whh8

        def load_attn_consts():
            for dc in range(ND):
                nc.sync.dma_start(xb[dc], io["xbt"].ap()[dc * P:(dc + 1) * P, :])
                nc.sync.dma_start(
                    lwb[dc], io["lwtb"].ap()[dc * P:(dc + 1) * P, :]
                )
            nc.sync.dma_start(
                lb_sb, io["lb"].ap().rearrange("(c o) -> c o", o=1)
            )

        with tc.tile_pool(name="gp", bufs=2, space="PSUM") as gp:

            def win_alloc(W):
                k, slot = W["k"], W["slot"]
                hw0 = W["w"] // 2
                # slot 0 (A) never rotates; slot 1 double-buffers for the
                # C -> B handoff
                p64t = ppool.tile([P, ND, 4, L], FP8, tag=f"P{slot}",
                                  bufs=(1 if slot == 0 else 2))
                W["P64"] = p64t
                c = []
                for dc in range(ND):
                    ct = state.tile([P, L], BF16, tag=f"c{slot}{dc}")
                    nc.gpsimd.memset(ct[:, 0:hw0], 0.0)
                    c.append(ct)
                W["c"] = c
                h8 = []
                for j in range(NJ):
                    ht = state.tile([P, 2, L], FP8, tag=f"h8{slot}{j}")
                    nc.gpsimd.memset(ht[:, :, 0:hw0], 0.0)
                    h8.append(ht)
                W["h8"] = h8
                loc = []
                for dc in range(ND):
                    lt = state.tile([P, L], BF16, tag=f"loc{k}_{dc}")
                    loc.append(lt)
                W["loc"] = loc
                locs[k] = loc

            def win_proj(W):
                # input projection + PSUM->SBUF fp8 conversion with bias
                k = W["k"]
                wih8 = W["wih8"]
                P64 = W["P64"]
                for dc in range(ND):
                    gm = _gate_ms(dc)
                    q = gp.tile([P, 4, L], F32, tag="q")
                    for gi in range(4):
                        m = gm[gi]
                        for j in range(NJ):
                            nc.tensor.matmul(
                                q[:, gi, :],
                                lhsT=wih8[j][:, :, m * P:(m + 1) * P],
                                rhs=x8[j][:, :, :],
                                start=(j == 0),
                                stop=(j == NJ - 1),
                                perf_mode=DR,
                                skip_group_check=True,
                            )
                    for gi in range(4):
                        m = gm[gi]
                        nc.vector.tensor_scalar_add(
                            P64[:, dc, gi, :], q[:, gi, :],
                            bias_sb[:, k, m:m + 1],
                        )

            def win_step0(W):
                # step 0: h == 0, gates straight from P64 (shifted); i and o
                # share one strided sigmoid (f unused at step 0)
                P64, c, h8 = W["P64"], W["c"], W["h8"]
                hw_ = W["w"] // 2
                s0, e0 = hw_, L
                n0 = e0 - s0
                off0 = -hw_
                for dc in range(ND):
                    g3 = gpool.tile([P, 3, L], BF16, tag="g3", bufs=4)
                    nc.scalar.activation(
                        g3[:, 0:3:2, :n0],
                        P64[:, dc, 0:3:2, s0 + off0:e0 + off0],
                        AF.Sigmoid, scale=inv_ws,
                    )
                    gt = gpool.tile([P, L], BF16, tag="gt", bufs=3)
                    nc.scalar.activation(
                        gt[:, :n0], P64[:, dc, 3, s0 + off0:e0 + off0],
                        AF.Tanh, scale=inv_ws,
                    )
                    nc.vector.tensor_mul(c[dc][:, s0:e0], g3[:, 0, :n0],
                                         gt[:, :n0])
                    tcz = gpool.tile([P, L], BF16, tag="tc", bufs=3)
                    nc.scalar.activation(tcz[:, :n0], c[dc][:, s0:e0], AF.Tanh)
                    nc.gpsimd.tensor_mul(
                        h8[dc // 2][:, dc % 2, s0:e0], g3[:, 2, :n0],
                        tcz[:, :n0],
                    )

            def step_ranges(W, t):
                hw_ = W["w"] // 2
                off = t - hw_
                s = max(0, -off)
                e = min(L, L - off)
                return off, s, e, e - s

            def mm_phase(W, t):
                off, s, e, n = step_ranges(W, t)
                P64, whh8, h8 = W["P64"], W["whh8"], W["h8"]
                qs = []
                for dc in range(ND):
                    gm = _gate_ms(dc)
                    q = gp.tile([P, 4, L], F32, tag="q")
                    qs.append(q)
                    # P64 re-injection first (no h8 dependency): DR ident
                    # selects one component of a P64 gate-pair view.  Order
                    # 0,2,1,3 so the two even-select loads are adjacent.
                    for gi in (0, 2, 1, 3):
                        pair = gi & ~1
                        nc.tensor.matmul(
                            q[:, gi, s:e],
                            lhsT=ident_sb[gi % 2],
                            rhs=P64[:, dc, pair:pair + 2, s + off:e + off],
                            start=True,
                            stop=False,
                            perf_mode=DR,
                            skip_group_check=True,
                        )
                    for gi in range(4):
                        m = gm[gi]
                        for j in range(NJ):
                            nc.tensor.matmul(
                                q[:, gi, :],
                                lhsT=whh8[j][:, :, m * P:(m + 1) * P],
                                rhs=h8[j][:, :, :],
                                start=False,
                                stop=(j == NJ - 1),
                                perf_mode=DR,
                                skip_group_check=True,
                            )
                W["qs"] = qs

            def tail_phase(W, t):
                off, s, e, n = step_ranges(W, t)
                hw_ = W["w"] // 2
                last = (t == W["w"] - 1)
                c, loc, qs, h8 = W["c"], W["loc"], W["qs"], W["h8"]
                g3s = {}
                # tanh(c) lags the gate pass by one d-chunk so the ACT queue
                # never head-of-line blocks on the DVE c-update.
                for dc in range(ND + 1):
                    if dc < ND:
                        g3 = gpool.tile([P, 3, L], BF16, tag="g3", bufs=4)
                        nc.scalar.activation(
                            g3[:, :, :n], qs[dc][:, 0:3, s:e], AF.Sigmoid,
                            scale=inv_ws,
                        )
                        g3s[dc] = g3
                        gt = gpool.tile([P, L], BF16, tag="gt", bufs=3)
                        nc.scalar.activation(
                            gt[:, :n], qs[dc][:, 3, s:e], AF.Tanh, scale=inv_ws
                        )
                        t1 = tmp.tile([P, L], BF16, tag="tt", bufs=3)
                        nc.vector.tensor_mul(t1[:, :n], g3[:, 0, :n], gt[:, :n])
                        t2 = tmp.tile([P, L], BF16, tag="tt", bufs=3)
                        nc.vector.tensor_mul(t2[:, :n], g3[:, 1, :n],
                                             c[dc][:, s:e])
                        nc.vector.tensor_add(c[dc][:, s:e], t1[:, :n], t2[:, :n])
                    if dc >= 1:
                        pd = dc - 1
                        g3p = g3s.pop(pd)
                        tcz = gpool.tile([P, L], BF16, tag="tc", bufs=3)
                        nc.scalar.activation(tcz[:, :n], c[pd][:, s:e], AF.Tanh)
                        if last:
                            # final h for all columns still pending
                            nc.vector.tensor_mul(
                                loc[pd][:, s:e], g3p[:, 2, :n], tcz[:, :n]
                            )
                        else:
                            # h8 fp8 computed directly on gpsimd
                            nc.gpsimd.tensor_mul(
                                h8[pd // 2][:, pd % 2, s:e], g3p[:, 2, :n],
                                tcz[:, :n],
                            )
                            if t >= hw_:
                                # column e-1 gets its final h at this step
                                nc.vector.tensor_mul(
                                    loc[pd][:, e - 1:e], g3p[:, 2, n - 1:n],
                                    tcz[:, n - 1:n],
                                )

            A, Bw, C = WIN["A"], WIN["B"], WIN["C"]
            win_dma_wih(A)
            win_dma_whh(A)
            win_alloc(A)
            win_proj(A)
            win_step0(A)
            win_dma_wih(C)
            win_dma_whh(C)
            load_attn_consts()
            win_alloc(C)
            win_proj(C)
            win_step0(C)

            # x-residual head matmul in the prologue, borrowing one PSUM
            # quad rotation slot (runs as soon as xb lands)
            qh = gp.tile([P, 4, L], F32, tag="q")
            ps_log = qh[0:NL, 0, :]
            for dc in range(ND):
                nc.tensor.matmul(
                    ps_log,
                    lhsT=lwb[dc][:],
                    rhs=xb[dc][:],
                    start=(dc == 0),
                    stop=(dc == ND - 1),
                )
            xhead = logp.tile([NL, L], F32, tag="xhead")
            nc.scalar.activation(xhead, ps_log, AF.Identity, bias=lb_sb[:, 0:1])

            # rounds: A steps 1..6; C steps 1..2; B steps 1..4 in rounds
            # 3..6 (proj/step0 B slot in after round 2, reusing C's slot)
            for r in range(1, 7):
                active = []
                active.append((A, r))
                if r <= 2:
                    active.append((C, r))
                if r >= 3:
                    active.append((Bw, r - 2))
                for W, t in active:
                    mm_phase(W, t)
                for W, t in active:
                    tail_phase(W, t)
                if r == 2:
                    win_dma_wih(Bw)
                    win_dma_whh(Bw)
                    win_alloc(Bw)
                    win_proj(Bw)
                    win_step0(Bw)

        locs = [locs[0], locs[1], locs[2]]
        # ---- attention over the 3 window outputs ----
        with tc.tile_pool(name="ap2", bufs=1, space="PSUM") as ap2:
            # dots: a_k[l] = sum_d x[d,l] * loc_k[d,l], bf16 muls + ones
            # reduction, all three into one [1, 3, L] PSUM tile
            psd3 = ap2.tile([1, NW, L], F32, tag="psd3")
            for k in (0, 1, 2):
                for dc in range(ND):
                    td = atmp.tile([P, L], BF16, tag="tf", bufs=2)
                    nc.vector.tensor_mul(td, xb[dc][:], locs[k][dc][:])
                    nc.tensor.matmul(
                        psd3[0:1, k, :],
                        lhsT=ones_col[:],
                        rhs=td[:],
                        start=(dc == 0),
                        stop=(dc == ND - 1),
                    )
            a3 = attn.tile([1, NW, L], BF16, tag="a3", bufs=1)
            nc.scalar.activation(a3[0:1, :, :], psd3[0:1, :, :], AF.Copy,
                                 scale=1.0 / math.sqrt(D))

            mx1 = attn.tile([1, L], F32, tag="mx", bufs=2)
            nc.vector.tensor_max(mx1, a3[0:1, 0, :], a3[0:1, 1, :])
            mx2 = attn.tile([1, L], F32, tag="mx", bufs=2)
            nc.vector.tensor_max(mx2, mx1[:], a3[0:1, 2, :])
            d3 = attn.tile([1, NW, L], BF16, tag="d3", bufs=1)
            for k in range(NW):
                nc.vector.tensor_sub(d3[0:1, k, :], a3[0:1, k, :], mx2[:])
            e3 = attn.tile([1, NW, L], BF16, tag="e3", bufs=1)
            nc.scalar.activation(e3[0:1, :, :], d3[0:1, :, :], AF.Exp)
            s1 = attn.tile([1, L], F32, tag="mx", bufs=2)
            nc.vector.tensor_add(s1, e3[0:1, 0, :], e3[0:1, 1, :])
            s2 = attn.tile([1, L], F32, tag="mx", bufs=2)
            nc.vector.tensor_add(s2, s1[:], e3[0:1, 2, :])
            r_ = attn.tile([1, L], F32, tag="mx", bufs=2)
            nc.vector.reciprocal(r_, s2[:])

            # attention weights broadcast to [NL, L] and per-window head
            # matmuls Y_k = lin_w @ loc_k; logits = xhead + sum_k wb_k * Y_k
            wb9 = []
            ys = []
            for k in range(NW):
                wk = attn.tile([1, NW, L], BF16, tag="wk", bufs=1)
                nc.vector.tensor_mul(wk[0:1, k, :], e3[0:1, k, :], r_[:])
                pb = ap2.tile([NL, L], F32, tag="bc", bufs=2)
                nc.tensor.matmul(
                    pb, lhsT=ones_row[0:1, 0:NL], rhs=wk[0:1, k, :],
                    start=True, stop=True,
                )
                wbs = attn.tile([NL, L], BF16, tag=f"wbs{k}", bufs=1)
                nc.scalar.activation(wbs, pb, AF.Copy)
                wb9.append(wbs)
                ps_y = ap2.tile([NL, L], F32, tag=f"y{k}")
                for dc in range(ND):
                    nc.tensor.matmul(
                        ps_y,
                        lhsT=lwb[dc][:],
                        rhs=locs[k][dc][:],
                        start=(dc == 0),
                        stop=(dc == ND - 1),
                    )
                ys.append(ps_y)

            # accumulate the weighted window terms into xhead in place
            for k in range(NW):
                uk = logp.tile([NL, L], F32, tag="u", bufs=2)
                nc.vector.tensor_mul(uk, ys[k][:], wb9[k][:])
                nc.vector.tensor_add(xhead, xhead[:], uk[:])
            nc.sync.dma_start(io["out"].ap().rearrange("l c -> c l"), xhead[:])


_NC_CACHE = {}


def _get_nc():
    if "nc" not in _NC_CACHE:
        nc = bacc.Bacc("TRN2", target_bir_lowering=False, debug=False)
        io = {
            "xbt": nc.dram_tensor("xbt", [D, L], BF16, kind="ExternalInput"),
            "x8": nc.dram_tensor("x8", [NJ, P, 2, L], FP8, kind="ExternalInput"),
            "wih8": nc.dram_tensor("wih8", [NW, NJ, P, 2, G4], FP8, kind="ExternalInput"),
            "whh8": nc.dram_tensor(
                "whh8", [NW, NJ, P, 2, G4], FP8, kind="ExternalInput"
            ),
            "bias64": nc.dram_tensor("bias64", [NW, G4], F32, kind="ExternalInput"),
            "lwtb": nc.dram_tensor("lwtb", [D, NL], BF16, kind="ExternalInput"),
            "lb": nc.dram_tensor("lb", [NL], F32, kind="ExternalInput"),
            "identp": nc.dram_tensor("identp", [2, P, 2, P], FP8, kind="ExternalInput"),
            "out": nc.dram_tensor("out", [L, NL], F32, kind="ExternalOutput"),
        }
        with tile.TileContext(nc) as tc:
            _emit(tc, io)
        nc.compile()
        _NC_CACHE["nc"] = nc
    return _NC_CACHE["nc"]


def _in_maps(sequence_output, W_ih, W_hh, b_ih, b_hh, lin_w, lin_b):
    x = np.asarray(sequence_output, np.float32)
    bf = ml_dtypes.bfloat16
    e4 = ml_dtypes.float8_e4m3
    WihT = np.transpose(np.asarray(W_ih, np.float32), (0, 2, 1))  # [NW, D, G4]
    Wih8 = np.clip(WihT * WS, -240.0, 240.0).astype(e4)
    Wih8 = np.ascontiguousarray(
        Wih8.reshape(NW, NJ, 2, P, G4).transpose(0, 1, 3, 2, 4)
    )
    WhhT = np.transpose(np.asarray(W_hh, np.float32), (0, 2, 1))  # [NW, D, G4]
    Whh8 = np.clip(WhhT * WS, -240.0, 240.0).astype(e4)
    # DoubleRow interleave: [NW, D, G4] -> [NW, NJ, P, 2, G4]
    Whh8 = np.ascontiguousarray(
        Whh8.reshape(NW, NJ, 2, P, G4).transpose(0, 1, 3, 2, 4)
    )
    # proj PSUM is already 64*(Wih@x) (weights pre-scaled), so bias ships x64
    bias64 = WS * (np.asarray(b_ih, np.float32) + np.asarray(b_hh, np.float32))
    lwt = np.ascontiguousarray(np.asarray(lin_w, np.float32).T)
    lb = np.asarray(lin_b, np.float32)
    # DR component-select identities
    identp = np.zeros((2, P, 2, P), np.float32)
    identp[0, np.arange(P), 0, np.arange(P)] = 1.0
    identp[1, np.arange(P), 1, np.arange(P)] = 1.0
    identp = identp.astype(e4)
    maps = []
    for b in range(B):
        xT = np.ascontiguousarray(x[b].T)
        x8 = np.clip(xT, -240.0, 240.0).astype(e4)
        x8 = np.ascontiguousarray(
            x8.reshape(NJ, 2, P, L).transpose(0, 2, 1, 3)
        )
        maps.append({
            "xbt": xT.astype(bf),
            "x8": x8,
            "wih8": Wih8,
            "whh8": Whh8,
            "bias64": bias64,
            "lwtb": lwt.astype(bf),
            "lb": lb,
            "identp": identp,
        })
    return maps


def kernel(sequence_output, W_ih, W_hh, b_ih, b_hh, lin_w, lin_b):
    nc = _get_nc()
    maps = _in_maps(sequence_output, W_ih, W_hh, b_ih, b_hh, lin_w, lin_b)
    res = bass_utils.run_bass_kernel_spmd(nc, maps, core_ids=list(range(N_CORES)))
    return np.stack([res.results[b]["out"] for b in range(B)], axis=0)


def run_traced(inputs, **kw):
    """For test.py: run with NTFF tracing, returns BassKernelResults."""
    nc = _get_nc()
    maps = _in_maps(**inputs)
    return bass_utils.run_bass_kernel_spmd(
        nc, maps, core_ids=list(range(N_CORES)), trace=True, **kw
    )
